# revision 17
# baseline (speedup 1.0000x reference)
"""Mesh2SDF Trainium2 kernel (8 NeuronCores, SPMD data-parallel over voxels).

Math (per point p, triangle T, all f32):
  d2(p,T) = inside(p,T) ? plane_dist2 : min over the 3 clamped edge segments
where
  - edge e anchored at q: D = 2*dot(p-q, e) (from PE GEMMs), t = clamp01(D*R'),
    cand = t*(t*|e|^2 - D) is the squared distance relative to |p-q|^2,
  - plane_dist2 = (dot(p, n_hat) - dot(a, n_hat))^2, gated by three half-plane
    tests W_i = cross(n, edge_i) via huge additive penalties,
  - the final d2 carries the triangle-side sign bit in its mantissa LSB so a
    pure f32 min-reduction (free dim on DVE, partition tree) selects both.
Per (128-triangle chunk x 512-point tile): 8 small K=4 fp32 GEMMs on PE,
~12 activations on ScalarE (PSUM->SBUF with fused per-partition scale/bias),
~17 tensor_tensor + ~6 tensor_scalar ops on VectorE.
"""
import os
import numpy as np

BIG = np.float32(1.0e30)
TINY = 1.0e-20
F32 = np.float32

N_CORES = 8
D_VOL = 64
M_TOTAL = D_VOL ** 3            # 262144 query points
M_CORE = M_TOTAL // N_CORES     # 32768 per core
MF = 512                        # points per tile (one PSUM bank)
CHUNK = 128                     # triangles per chunk (partition dim)

# ACTC slots (per-partition scalars for ScalarE, one [128] vector per chunk)
A_DR1, A_DR2, A_DR3, A_D3R, A_S2, A_AP2, A_P1, A_P2, A_P3, A_AN = range(10)
# TSC slots (per-partition scalars for VectorE tensor_scalar)
T_RAB, T_RAC, T_RBC, T_E2AB, T_E2AC, T_E2BC = range(6)
# GEMM direction slots
G_AB, G_AC, G_BC, G_A, G_N, G_W1, G_W2, G_W3 = range(8)


def host_consts(vertices, faces):
    """Per-triangle constants. Returns wt (4,S,8,128), pbw (4,128),
    actc (128,S,10), tsc (128,S,6) with S = F//CHUNK chunks."""
    v = np.asarray(vertices, F32)
    f = np.asarray(faces)
    F = f.shape[0]
    S = F // CHUNK
    a, b, c = v[f[:, 0]], v[f[:, 1]], v[f[:, 2]]
    ab, ac, bc = b - a, c - a, c - b
    n = np.cross(ab, ac).astype(F32)
    nn = (n * n).sum(1)
    good = nn > TINY
    nhat = np.where(good[:, None],
                    n / np.sqrt(np.maximum(nn, TINY))[:, None], 0.0).astype(F32)
    dot = lambda u, w: (u * w).sum(1).astype(F32)
    e2ab, e2ac, e2bc = dot(ab, ab), dot(ac, ac), dot(bc, bc)
    rcp = lambda e2: np.where(e2 > TINY, 1.0 / (2.0 * e2), 0.0).astype(F32)
    W1 = np.cross(n, ab).astype(F32)
    W2 = np.cross(n, bc).astype(F32)
    W3 = np.cross(n, a - c).astype(F32)
    K1, K2, K3 = dot(a, W1), dot(b, W2), dot(c, W3)
    W1[~good] = 0; W2[~good] = 0; W3[~good] = 0
    K1[~good] = 1.0; K2[~good] = 1.0; K3[~good] = 1.0
    An = dot(a, nhat)

    wt = np.zeros((4, S, 8, CHUNK), F32)
    dirs = {G_AB: ab, G_AC: ac, G_BC: bc, G_A: a, G_N: nhat,
            G_W1: W1, G_W2: W2, G_W3: W3}
    for j, e in dirs.items():
        wt[:3, :, j, :] = e.T.reshape(3, S, CHUNK)
    pbw = np.zeros((4, CHUNK), F32)
    pbw[3, :] = 1.0

    actc = np.zeros((CHUNK, S, 10), F32)
    tsc = np.zeros((CHUNK, S, 6), F32)

    def put(arr, slot, vals):
        arr[:, :, slot] = np.asarray(vals, F32).reshape(S, CHUNK).T

    put(actc, A_DR1, -2.0 * dot(a, ab))
    put(actc, A_DR2, -2.0 * dot(a, ac))
    put(actc, A_DR3, -2.0 * dot(b, bc))
    put(actc, A_D3R, -2.0 * dot(b, ab))
    put(actc, A_S2, -An)
    put(actc, A_AP2, dot(a, a))
    put(actc, A_P1, BIG * K1)
    put(actc, A_P2, BIG * K2)
    put(actc, A_P3, BIG * K3)
    put(actc, A_AN, An)
    put(tsc, T_RAB, rcp(e2ab))
    put(tsc, T_RAC, rcp(e2ac))
    put(tsc, T_RBC, rcp(e2bc))
    put(tsc, T_E2AB, e2ab)
    put(tsc, T_E2AC, e2ac)
    put(tsc, T_E2BC, e2bc)
    return wt, pbw, actc, tsc


def build_nc(n_points=M_CORE, n_chunks=16, mf=MF):
    """Build the Bass program (one core's SPMD program)."""
    import concourse.bass as bass
    import concourse.mybir as mybir
    import concourse.tile as tile
    from concourse import bacc
    from concourse.bass import ds

    f32 = mybir.dt.float32
    i32 = mybir.dt.int32
    AF = mybir.ActivationFunctionType
    OP = mybir.AluOpType
    nc = bacc.Bacc()

    pt4 = nc.declare_dram_parameter("pt4", [4, n_points], f32, isOutput=False)
    wtd = nc.declare_dram_parameter("wt", [4, n_chunks, 8, CHUNK], f32, isOutput=False)
    pbwd = nc.declare_dram_parameter("pbw", [4, CHUNK], f32, isOutput=False)
    actcd = nc.declare_dram_parameter("actc", [CHUNK, n_chunks, 10], f32, isOutput=False)
    tscd = nc.declare_dram_parameter("tsc", [CHUNK, n_chunks, 6], f32, isOutput=False)
    outd = nc.declare_dram_parameter("out", [n_points], f32, isOutput=True)

    with tile.TileContext(nc) as tc:
        with (
            tc.tile_pool(name="singles", bufs=1) as singles,
            tc.tile_pool(name="ptp", bufs=2) as ptp,
            tc.tile_pool(name="psum", bufs=8, space="PSUM") as psum,
            tc.tile_pool(name="work", bufs=2) as work,
            tc.tile_pool(name="acc", bufs=2) as accp,
            tc.tile_pool(name="wpool", bufs=3) as wpool,
        ):
            pbw = singles.tile([4, CHUNK], f32)
            nc.sync.dma_start(out=pbw, in_=pbwd[:])
            actc = singles.tile([CHUNK, n_chunks, 10], f32)
            nc.sync.dma_start(out=actc, in_=actcd[:])
            tsc = singles.tile([CHUNK, n_chunks, 6], f32)
            nc.sync.dma_start(out=tsc, in_=tscd[:])

            def ac(c, k):
                return actc[:, c, k:k + 1]

            def ts(c, k):
                return tsc[:, c, k:k + 1]

            def body(moff):
                rhs = ptp.tile([4, mf], f32)
                nc.sync.dma_start(out=rhs, in_=pt4[:, ds(moff, mf)])
                # p2 broadcast across partitions: ones-row matmul, then to SBUF
                psP = psum.tile([CHUNK, mf], f32, tag="ps")
                nc.tensor.matmul(psP, pbw[:], rhs[:], start=True, stop=True)
                p2b = accp.tile([CHUNK, mf], f32)
                nc.scalar.copy(p2b, psP)
                # best holds the running MAX of (-d2 with sign-flag LSB); the
                # negation is folded into the pack (XOR of the f32 sign bit)
                # so the final partition reduction can use GPSIMD all-reduce
                # (which supports max but not min).
                best = accp.tile([CHUNK, mf], f32)
                nc.vector.memset(best, -3.0e38)

                for c in range(n_chunks):
                    wtc = wpool.tile([4, 8, CHUNK], f32)
                    nc.sync.dma_start(out=wtc, in_=wtd[:, c, :, :])
                    ps = []
                    for j in range(8):
                        pj = psum.tile([CHUNK, mf], f32, tag="ps")
                        nc.tensor.matmul(pj, wtc[:, j, :], rhs[:],
                                         start=True, stop=True)
                        ps.append(pj)
                    x, y, wv, z, s, g1, g2, g3 = ps

                    dr1 = work.tile([CHUNK, mf], f32)
                    nc.scalar.activation(dr1, x, AF.Relu, bias=ac(c, A_DR1), scale=2.0)
                    dr2 = work.tile([CHUNK, mf], f32)
                    nc.scalar.activation(dr2, y, AF.Relu, bias=ac(c, A_DR2), scale=2.0)
                    dr3 = work.tile([CHUNK, mf], f32)
                    nc.scalar.activation(dr3, wv, AF.Relu, bias=ac(c, A_DR3), scale=2.0)
                    d3r = work.tile([CHUNK, mf], f32)
                    nc.scalar.activation(d3r, x, AF.Identity, bias=ac(c, A_D3R), scale=2.0)
                    s2 = work.tile([CHUNK, mf], f32)
                    nc.scalar.activation(s2, s, AF.Square, bias=ac(c, A_S2), scale=1.0)
                    ap2 = work.tile([CHUNK, mf], f32)
                    nc.scalar.activation(ap2, z, AF.Identity, bias=ac(c, A_AP2), scale=-2.0)
                    pen1 = work.tile([CHUNK, mf], f32)
                    nc.scalar.activation(pen1, g1, AF.Relu, bias=ac(c, A_P1), scale=-BIG)
                    pen2 = work.tile([CHUNK, mf], f32)
                    nc.scalar.activation(pen2, g2, AF.Relu, bias=ac(c, A_P2), scale=-BIG)
                    pen3 = work.tile([CHUNK, mf], f32)
                    nc.scalar.activation(pen3, g3, AF.Relu, bias=ac(c, A_P3), scale=-BIG)
                    # sign of side (side = s - An < 0), as int 0/1
                    sgf = work.tile([CHUNK, mf], f32)
                    nc.vector.tensor_scalar(sgf, s, ac(c, A_AN), None, OP.is_lt)
                    sgi = work.tile([CHUNK, mf], i32)
                    nc.vector.tensor_copy(sgi, sgf)

                    # edge candidates: tc = min(Dr*R',1); u = tc*e2;
                    # cand = tc*(u - Dr)
                    tc1 = work.tile([CHUNK, mf], f32)
                    nc.vector.tensor_scalar(tc1, dr1, ts(c, T_RAB), 1.0, OP.mult, OP.min)
                    tc2 = work.tile([CHUNK, mf], f32)
                    nc.vector.tensor_scalar(tc2, dr2, ts(c, T_RAC), 1.0, OP.mult, OP.min)
                    tc3 = work.tile([CHUNK, mf], f32)
                    nc.vector.tensor_scalar(tc3, dr3, ts(c, T_RBC), 1.0, OP.mult, OP.min)
                    u1 = work.tile([CHUNK, mf], f32)
                    nc.scalar.mul(u1, tc1, ts(c, T_E2AB))
                    u2 = work.tile([CHUNK, mf], f32)
                    nc.scalar.mul(u2, tc2, ts(c, T_E2AC))
                    u3 = work.tile([CHUNK, mf], f32)
                    nc.scalar.mul(u3, tc3, ts(c, T_E2BC))
                    h1 = work.tile([CHUNK, mf], f32)
                    nc.vector.tensor_tensor(h1, u1, dr1, OP.subtract)
                    m1 = work.tile([CHUNK, mf], f32)
                    nc.vector.tensor_tensor(m1, tc1, h1, OP.mult)
                    h2 = work.tile([CHUNK, mf], f32)
                    nc.vector.tensor_tensor(h2, u2, dr2, OP.subtract)
                    m2 = work.tile([CHUNK, mf], f32)
                    nc.vector.tensor_tensor(m2, tc2, h2, OP.mult)
                    h3 = work.tile([CHUNK, mf], f32)
                    nc.vector.tensor_tensor(h3, u3, dr3, OP.subtract)
                    m3 = work.tile([CHUNK, mf], f32)
                    nc.vector.tensor_tensor(m3, tc3, h3, OP.mult)
                    c1 = work.tile([CHUNK, mf], f32)
                    nc.vector.tensor_scalar(c1, m3, ts(c, T_E2AB), None, OP.subtract)
                    cbc = work.tile([CHUNK, mf], f32)
                    nc.vector.tensor_tensor(cbc, c1, d3r, OP.subtract)
                    em = work.tile([CHUNK, mf], f32)
                    nc.vector.tensor_tensor(em, m1, m2, OP.min)
                    em2 = work.tile([CHUNK, mf], f32)
                    nc.vector.tensor_tensor(em2, em, cbc, OP.min)
                    aps = work.tile([CHUNK, mf], f32)
                    nc.vector.tensor_tensor(aps, p2b, ap2, OP.add)
                    d2o = work.tile([CHUNK, mf], f32)
                    nc.vector.tensor_tensor(d2o, em2, aps, OP.add)
                    pp = work.tile([CHUNK, mf], f32)
                    nc.vector.tensor_tensor(pp, pen1, pen2, OP.add)
                    qq = work.tile([CHUNK, mf], f32)
                    nc.vector.tensor_tensor(qq, s2, pen3, OP.add)
                    d2i = work.tile([CHUNK, mf], f32)
                    nc.vector.tensor_tensor(d2i, pp, qq, OP.add)
                    d2 = work.tile([CHUNK, mf], f32)
                    nc.vector.tensor_tensor(d2, d2o, d2i, OP.min)
                    # negate (XOR sign bit), clear mantissa LSB, insert side
                    # flag, fold into running max
                    pka = work.tile([CHUNK, mf], i32)
                    nc.vector.tensor_scalar(pka, d2.bitcast(i32), -2147483648,
                                            -2, OP.bitwise_xor, OP.bitwise_and)
                    pk = work.tile([CHUNK, mf], i32)
                    nc.vector.tensor_tensor(pk, pka, sgi, OP.bitwise_or)
                    nc.vector.tensor_tensor(best, best, pk.bitcast(f32), OP.max)

                # partition tree-max: DMA shifts halves to partition 0 so all
                # compute APs share a start partition, then a 32x32 block
                # transpose + free-dim reduce finishes 32 -> 1.
                tmp = accp.tile([64, mf], f32)
                nc.sync.dma_start(out=tmp, in_=best[64:128, :])
                nc.vector.tensor_tensor(best[0:64, :], best[0:64, :],
                                        tmp[:, :], OP.max)
                tmp2 = accp.tile([32, mf], f32)
                nc.sync.dma_start(out=tmp2, in_=best[32:64, :])
                nc.vector.tensor_tensor(best[0:32, :], best[0:32, :],
                                        tmp2[:, :], OP.max)
                bt = accp.tile([32, mf], f32)
                nc.vector.transpose(bt, best[0:32, :])
                red = accp.tile([32, mf // 32], f32)
                nc.vector.tensor_reduce(red, bt.rearrange("p (k j) -> p k j", j=32),
                                        axis=mybir.AxisListType.X, op=OP.max)
                nc.sync.dma_start(
                    out=outd[ds(moff, mf)].rearrange("(k p) -> p k", p=32),
                    in_=red)

            n_mt = n_points // mf
            if n_mt == 1:
                body(0)
            else:
                with tc.For_i(0, n_points, mf, staggered_reset=True) as moff:
                    body(moff)
    return nc


def _host_prep(vertices, faces, volume_size):
    D = int(volume_size)
    assert D == D_VOL, f"kernel hardcoded for volume_size=64, got {D}"
    lin = np.linspace(-1.0, 1.0, D, dtype=F32)
    gx, gy, gz = np.meshgrid(lin, lin, lin, indexing="ij")
    P = np.stack([gx.ravel(), gy.ravel(), gz.ravel()], -1).astype(F32)
    p2 = (P * P).sum(1).astype(F32)
    pt4 = np.concatenate([P.T, p2[None, :]], axis=0).astype(F32)  # (4, M)
    wt, pbw, actc, tsc = host_consts(vertices, faces)
    return pt4, wt, pbw, actc, tsc


def _postprocess(packed):
    """packed: (M,) f32 = -(d2 with side flag in mantissa LSB)."""
    bits = packed.view(np.int32)
    sign = np.where(bits & 1 != 0, F32(-1), F32(1))
    d2 = np.maximum(-packed, F32(0))
    return (sign * np.sqrt(d2)).reshape(D_VOL, D_VOL, D_VOL)


_LAST_RESULTS = None


_NC_CACHE = None


def kernel(vertices, faces, volume_size):
    global _LAST_RESULTS, _NC_CACHE
    import time as _time
    from concourse.bass_utils import run_bass_kernel_spmd

    t0 = _time.time()
    pt4, wt, pbw, actc, tsc = _host_prep(vertices, faces, volume_size)
    t1 = _time.time()
    if _NC_CACHE is None:
        nc = build_nc()
        nc.finalize()
        _NC_CACHE = nc
    nc = _NC_CACHE
    t2 = _time.time()
    in_maps = []
    for k in range(N_CORES):
        in_maps.append({
            "pt4": np.ascontiguousarray(pt4[:, k * M_CORE:(k + 1) * M_CORE]),
            "wt": wt, "pbw": pbw, "actc": actc, "tsc": tsc,
        })
    trace = bool(int(os.environ.get("MESH2SDF_TRACE", "0")))
    res = run_bass_kernel_spmd(nc, in_maps, list(range(N_CORES)), trace=trace)
    t3 = _time.time()
    if os.environ.get("MESH2SDF_PHASES"):
        res2 = run_bass_kernel_spmd(nc, in_maps, list(range(N_CORES)), trace=trace)
        t4 = _time.time()
        print(f"[kernel phases] host_prep {t1-t0:.3f}s build {t2-t1:.3f}s "
              f"run1 {t3-t2:.3f}s run2 {t4-t3:.3f}s")
    _LAST_RESULTS = res
    packed = np.concatenate([res.results[k]["out"] for k in range(N_CORES)])
    return _postprocess(packed)
